# revision 25
# baseline (speedup 1.0000x reference)
"""StyleGAN2-style modulated 3x3 conv (B=8, Ci=Co=512, H=W=32) on 8 TRN2 NeuronCores.

Sharding: data-parallel over batch, one sample per core (embarrassingly
parallel, no collectives). Per core the conv uses 1-D Winograd F(2,3)
along H: 4 "vertical frequency" streams replace the 3 kh taps, cutting
PE matmul columns 1.5x (192 matmuls of 512 cols vs 288 direct). Per
(co-chunk, vf) group, 12 matmuls (3 kw shifts x 4 ci chunks) accumulate
M[vf] in PSUM; ACT copies M out with the demod scale folded in; DVE
applies the 2-tap inverse transform (with bias folded via
scalar_tensor_tensor) to produce output rows 2t / 2t+1.

Math (per sample b, with s = (Ci*K*K)**-0.5 folded out of both the conv
and the demod norm so the weights can be used unscaled):
  conv = conv2d(x * y_s, weight)                     # raw, no s
  xs2[o] = sum_i y_s[i]^2 * w2[i,o],  w2 = sum_k weight[o,i,k]^2
  out = conv / sqrt(xs2 + 1e-8 * Ci * K * K) + bias

Winograd per (ci, kw):  g = weight[:, :, :, kw] taps over kh
  u0 = g0, u1 = (g0+g1+g2)/2, u2 = (g0-g1+g2)/2, u3 = g2     (host, free)
  v0 = d0-d2, v1 = d1+d2, v2 = d2-d1, v3 = d1-d3              (DVE)
  m_vf = sum_{ci,kw} u_vf . v_vf(shifted kw)                  (PE)
  out[2t]   = m0+m1+m2,  out[2t+1] = m1-m2-m3                 (DVE)

Host-side prep is layout only (zero-padding included) plus the
input-independent weight transforms (Winograd u slots + w2 fold);
all input-dependent math runs on device.
"""

import numpy as np
import ml_dtypes

import concourse.mybir as mybir
from concourse import bacc
from concourse.tile import TileContext
from concourse.bass_utils import run_bass_kernel_spmd

B = 8
CI = 512
CO = 512
H = W = 32
KK = 9  # 3x3
NCI = CI // 128
NCO = CO // 128
HP = 34  # padded H (and W) in DRAM
NSLOT = 13  # 12 winograd (vf*3+kw) + 1 w2
EPS_EFF = 1e-8 * CI * KK  # demod eps compensated for unscaled weights

F32 = mybir.dt.float32
BF16 = mybir.dt.bfloat16
AF = mybir.ActivationFunctionType
ALU = mybir.AluOpType


def build_nc():
    nc = bacc.Bacc("TRN2", target_bir_lowering=False, debug=False)

    # zero-padded (34x34), unmodulated input
    x_ext = nc.declare_dram_parameter("x", [NCI, 128, HP, HP], BF16, isOutput=False)
    # cols 0..3 = y_s per ci-tile, cols 4..7 = bias per co-tile
    yb_ext = nc.declare_dram_parameter("yb", [128, 2 * NCI], F32, isOutput=False)
    # w2 = sum_k w^2, [ci_p, jci, co] (one small transfer, needed early)
    wt2_ext = nc.declare_dram_parameter(
        "wt2", [128, NCI, CO], BF16, isOutput=False
    )
    # winograd slots grouped per (jo, vf): [jo, vf, ci_p, kw, jci, co];
    # one 2D-contiguous transfer per (jo, vf) = consumption granularity
    wt_ext = nc.declare_dram_parameter(
        "wt", [NCO, 4, 128, 3, NCI, 128], BF16, isOutput=False
    )
    # [jo, co_p, t, dy, w] -> h = 2t+dy enumerates in order, so this is
    # [jo, co_p, 32, 32] row-major
    out_ext = nc.declare_dram_parameter("out", [NCO, 128, H * W], F32, isOutput=True)

    with TileContext(nc) as tc:
        with (
            tc.tile_pool(name="singles", bufs=1) as singles,
            tc.tile_pool(name="wts", bufs=1) as wts,
            tc.tile_pool(name="xin", bufs=1) as xin,
            tc.tile_pool(name="vts", bufs=1) as vts,
            tc.tile_pool(name="msb", bufs=2) as msb,
            tc.tile_pool(name="stb", bufs=2) as stb,
            tc.tile_pool(name="outs", bufs=2) as outs,
            tc.tile_pool(name="cps", bufs=6, space="PSUM") as cps,
            tc.tile_pool(name="dps", bufs=1, space="PSUM") as dps,
            tc.tile_pool(name="wps", bufs=1, space="PSUM") as wps,
        ):
            # ---- input DMAs ----
            # x chunks serially on the sync queue (first-needed); weights on
            # the gpsimd queue (the ~0.7us/issue DGE cost lands on an
            # otherwise-idle engine): w2 first (demod), then (jo, vf) groups
            # in consumption order. The two queues share the DMA ring
            # round-robin, so x finishes ~halfway through the weight stream.
            yb_sb = singles.tile([128, 2 * NCI], F32)
            nc.sync.dma_start(out=yb_sb, in_=yb_ext[:, :])
            # ALL bulk input (x + weights) goes through the single gpsimd
            # SWDGE queue in consumption order: the software queue keeps many
            # descriptors in flight and sustains ~270 GB/s, while the HW-DGE
            # queues (sync/scalar) crawl at <100 GB/s under contention.
            # Serial order on one fast queue beats parallel slow queues.
            xt_sb = []
            for j in range(NCI):
                xt = xin.tile([128, HP, HP], BF16, tag=f"x{j}", name=f"xt{j}")
                xt_sb.append(xt)
            wt2_sb = wts.tile([128, NCI, CO], BF16, tag="wt2")
            wt_sb = [[None] * 4 for _ in range(NCO)]
            for jo in range(NCO):
                for vf in range(4):
                    wt_sb[jo][vf] = wts.tile(
                        [128, 3, NCI, 128], BF16, tag=f"wt{jo}_{vf}",
                        name=f"wt{jo}_{vf}",
                    )
            nc.gpsimd.dma_start(out=xt_sb[0], in_=x_ext[0])
            nc.gpsimd.dma_start(out=xt_sb[1], in_=x_ext[1])
            nc.gpsimd.dma_start(out=wt_sb[0][0], in_=wt_ext[0, 0])
            nc.gpsimd.dma_start(out=xt_sb[2], in_=x_ext[2])
            nc.gpsimd.dma_start(out=xt_sb[3], in_=x_ext[3])
            nc.gpsimd.dma_start(out=wt_sb[0][1], in_=wt_ext[0, 1])
            nc.gpsimd.dma_start(out=wt2_sb, in_=wt2_ext[:, :, :])
            for jo in range(NCO):
                for vf in range(4):
                    if jo == 0 and vf < 2:
                        continue
                    nc.gpsimd.dma_start(out=wt_sb[jo][vf], in_=wt_ext[jo, vf])

            # ---- PE warm-up: throwaway matmuls on memset data so the
            # HAM clock gate starts releasing before the real stream ----
            warm_lhs = singles.tile([128, 1], BF16)
            nc.vector.memset(warm_lhs, 1.0)
            warm_rhs = singles.tile([128, 512], BF16)
            nc.vector.memset(warm_rhs, 0.5)
            warm_ps = wps.tile([1, 512], F32)
            N_WARM = 6
            for i in range(N_WARM):
                nc.tensor.matmul(
                    out=warm_ps,
                    lhsT=warm_lhs,
                    rhs=warm_rhs,
                    start=(i == 0),
                    stop=(i == N_WARM - 1),
                )

            eps_sb = singles.tile([128, 1], F32)
            nc.vector.memset(eps_sb, EPS_EFF)

            # ---- modulate x by y_s (in place) and winograd-transform ----
            # vf-plane-major order after plane 0: conv group (0, vf) only
            # needs plane vf of every chunk, so the PE can start on plane 0
            # while later planes transform.
            vt_sb = []
            for j in range(NCI):
                vt_sb.append(
                    vts.tile([128, 4, 16, HP], BF16, tag=f"v{j}", name=f"vt{j}")
                )

            def vop(j, vf):
                d0 = xt_sb[j][:, 0:32:2, :]
                d1 = xt_sb[j][:, 1:33:2, :]
                d2 = xt_sb[j][:, 2:34:2, :]
                d3 = xt_sb[j][:, 3:34:2, :]
                v = vt_sb[j]
                if vf == 0:
                    nc.vector.tensor_sub(v[:, 0], d0, d2)
                elif vf == 1:
                    nc.vector.tensor_add(v[:, 1], d1, d2)
                elif vf == 2:
                    nc.vector.tensor_sub(v[:, 2], d2, d1)
                else:
                    nc.vector.tensor_sub(v[:, 3], d1, d3)

            JORD = (0, 1, 2, 3)  # matches x arrival order on the SWDGE queue
            for j in JORD:
                nc.vector.tensor_scalar(
                    out=xt_sb[j],
                    in0=xt_sb[j],
                    scalar1=yb_sb[:, j : j + 1],
                    scalar2=None,
                    op0=ALU.mult,
                )
                vop(j, 0)
            for vf in range(1, 4):
                for j in JORD:
                    vop(j, vf)
            # ys^2 in bf16 for the demod matmuls
            ys2_sb = singles.tile([128, NCI], BF16)
            nc.vector.tensor_mul(ys2_sb, yb_sb[:, 0:NCI], yb_sb[:, 0:NCI])

            xs2_ps = dps.tile([128, NCO], F32)
            rs_sb = singles.tile([128, NCO], F32)

            def conv_mms(jo, vf):
                ps = cps.tile([128, 512], F32, tag="ps")
                idx = 0
                for j in JORD:
                    for kw in range(3):
                        nc.tensor.matmul(
                            out=ps,
                            lhsT=wt_sb[jo][vf][:, kw, j, :],
                            rhs=vt_sb[j][:, vf, :, kw : kw + W],
                            start=(idx == 0),
                            stop=(idx == 11),
                        )
                        idx += 1
                return ps

            def demod_mms():
                for jo in range(NCO):
                    for j in range(NCI):
                        nc.tensor.matmul(
                            out=xs2_ps[:, jo : jo + 1],
                            lhsT=wt2_sb[:, j, jo * 128 : (jo + 1) * 128],
                            rhs=ys2_sb[:, j : j + 1],
                            start=(j == 0),
                            stop=(j == NCI - 1),
                        )

            def group_copy(ps, m, jo, vf, lo=0, hi=512):
                # PSUM -> SBUF with demod scale folded in
                nc.scalar.activation(
                    out=m[:, vf, lo:hi],
                    in_=ps[:, lo:hi],
                    func=AF.Identity,
                    scale=rs_sb[:, jo : jo + 1],
                )

            def filler(n):
                # dummy matmuls on memset data: keep the PE duty cycle up
                # during data stalls in the first ~25us so the HAM clock
                # gate does not demote back to half rate
                for _ in range(n):
                    nc.tensor.matmul(
                        out=warm_ps, lhsT=warm_lhs, rhs=warm_rhs,
                        start=True, stop=True,
                    )

            # out rows 2t   = m0+m1+m2 + bias
            # out rows 2t+1 = m1-m2-m3 + bias
            # partial inverse ops interleave with the ACT copies so only one
            # scalar_tensor_tensor trails the last copy of each co-chunk
            def inv_s0(m, jo, lo=0, hi=512):
                s0 = stb.tile([128, 512], BF16, tag=f"s0_{jo}")
                nc.vector.tensor_add(s0[:, lo:hi], m[:, 0, lo:hi], m[:, 1, lo:hi])
                return s0

            def inv_out0(m, s0, jo, ot, lo=0, hi=512):
                bcol = yb_sb[:, NCI + jo : NCI + jo + 1]
                nc.vector.scalar_tensor_tensor(
                    out=ot[:, lo // 32 : hi // 32, 0, :],
                    in0=s0[:, lo:hi],
                    scalar=bcol,
                    in1=m[:, 2, lo:hi],
                    op0=ALU.add,
                    op1=ALU.add,
                )

            def inv_s1(m, jo, lo=0, hi=512):
                s1 = stb.tile([128, 512], BF16, tag=f"s1_{jo}")
                nc.vector.tensor_sub(s1[:, lo:hi], m[:, 1, lo:hi], m[:, 2, lo:hi])
                return s1

            def inv_out1(m, s1, jo, ot, lo=0, hi=512):
                bcol = yb_sb[:, NCI + jo : NCI + jo + 1]
                nc.vector.scalar_tensor_tensor(
                    out=ot[:, lo // 32 : hi // 32, 1, :],
                    in0=s1[:, lo:hi],
                    scalar=bcol,
                    in1=m[:, 3, lo:hi],
                    op0=ALU.add,
                    op1=ALU.subtract,
                )

            # NOTE: emission order IS dataflow order under Tile. jo=0 groups
            # first (consuming V planes as they transform), demod after
            # conv(1,0) (w2 arrives mid-stream; rs is only needed by the
            # first ACT copy, which the 6-deep PSUM pool lets run late).
            mt = [None] * NCO
            ott = [None] * NCO
            pss = {}
            pss[(0, 0)] = conv_mms(0, 0)
            filler(4)
            pss[(0, 1)] = conv_mms(0, 1)
            filler(2)
            demod_mms()
            nc.scalar.activation(out=rs_sb, in_=xs2_ps, func=AF.Sqrt, bias=eps_sb)
            nc.vector.reciprocal(out=rs_sb, in_=rs_sb)
            pss[(0, 2)] = conv_mms(0, 2)
            pss[(0, 3)] = conv_mms(0, 3)
            pss[(1, 0)] = conv_mms(1, 0)

            def epilogue(jo):
                m, ot = mt[jo], ott[jo]
                group_copy(pss[(jo, 0)], m, jo, 0)
                group_copy(pss[(jo, 1)], m, jo, 1)
                s0 = inv_s0(m, jo)
                group_copy(pss[(jo, 2)], m, jo, 2)
                inv_out0(m, s0, jo, ot)
                s1 = inv_s1(m, jo)
                if jo == NCO - 1:
                    # split the tail: copy/inverse/DMA in halves so the
                    # last-half DMA overlaps the first-half compute
                    # halves on two different queues so the final two output
                    # DMAs overlap
                    group_copy(pss[(jo, 3)], m, jo, 3, 0, 256)
                    inv_out1(m, s1, jo, ot, 0, 256)
                    nc.sync.dma_start(out=out_ext[jo, :, 0:512], in_=ot[:, 0:8])
                    group_copy(pss[(jo, 3)], m, jo, 3, 256, 512)
                    inv_out1(m, s1, jo, ot, 256, 512)
                    nc.gpsimd.dma_start(
                        out=out_ext[jo, :, 512:1024], in_=ot[:, 8:16]
                    )
                else:
                    group_copy(pss[(jo, 3)], m, jo, 3)
                    inv_out1(m, s1, jo, ot)
                    nc.sync.dma_start(out=out_ext[jo], in_=ot)

            for jo in range(NCO):
                mt[jo] = msb.tile([128, 4, 512], BF16, tag="m", name=f"m{jo}")
                ott[jo] = outs.tile(
                    [128, 16, 2, W], F32, tag="ot", name=f"ot{jo}"
                )

            for vf in range(1, 4):
                pss[(1, vf)] = conv_mms(1, vf)
            epilogue(0)
            for vf in range(4):
                pss[(2, vf)] = conv_mms(2, vf)
            epilogue(1)
            for vf in range(4):
                pss[(3, vf)] = conv_mms(3, vf)
            epilogue(2)
            epilogue(3)

            # keep the warm-up matmuls live (cheap PSUM read at the end)
            warm_sink = singles.tile([1, 1], F32)
            nc.vector.tensor_copy(out=warm_sink, in_=warm_ps[0:1, 0:1])
    nc.compile()
    return nc


_NC_CACHE = None


def _get_nc():
    global _NC_CACHE
    if _NC_CACHE is None:
        _NC_CACHE = build_nc()
    return _NC_CACHE


def _prep_inputs(x, y_s, weight, bias):
    # winograd slots u[vf] over kh taps (exact in f64; /2 is exact in bf16)
    # -> [jo, vf, ci_p, kw, jci, co] bf16; w2 = sum_k w^2 -> [ci_p, jci, co]
    g = weight.astype(np.float64).transpose(2, 3, 1, 0)  # [kh, kw, ci, co]
    u = np.stack(
        [g[0], (g[0] + g[1] + g[2]) * 0.5, (g[0] - g[1] + g[2]) * 0.5, g[2]]
    ).astype(np.float32)  # [vf, kw, ci, co]
    wtq = np.ascontiguousarray(
        u.reshape(4, 3, NCI, 128, NCO, 128).transpose(4, 0, 3, 1, 2, 5)
    ).astype(ml_dtypes.bfloat16)
    w2 = (g ** 2).sum(axis=(0, 1)).astype(np.float32)  # [ci, co]
    wt2 = np.ascontiguousarray(
        w2.reshape(NCI, 128, CO).transpose(1, 0, 2)
    ).astype(ml_dtypes.bfloat16)
    in_maps = []
    for b in range(B):
        yb = np.empty((128, 2 * NCI), np.float32)
        yb[:, :NCI] = y_s[b].reshape(NCI, 128).T
        yb[:, NCI:] = bias.reshape(NCO, 128).T
        xp = np.zeros((NCI, 128, HP, HP), np.float32)
        xp[:, :, 1 : H + 1, 1 : W + 1] = x[b].reshape(NCI, 128, H, W)
        in_maps.append(
            {
                "x": xp.astype(ml_dtypes.bfloat16),
                "yb": yb,
                "wt": wtq,
                "wt2": wt2,
            }
        )
    return in_maps


def _install_trace_support():
    """Dev-only: register the axon NTFF profiling hook + disable the
    remote artifact upload so trace=True works in this container."""
    import sys
    import types

    import concourse.bass_utils as bu

    bu.upload_artifacts = lambda tmpdir: "local://" + str(tmpdir)
    if "antenv.axon_hooks" in sys.modules:
        return
    try:
        from trn_agent_boot.trn_boot import _ntff_profile_via_ctypes

        hook = _ntff_profile_via_ctypes("/opt/axon/libaxon_pjrt.so")
    except Exception:
        return
    mod = types.ModuleType("antenv.axon_hooks")
    mod.get_axon_ntff_profile_hook = lambda: hook
    mod.set_axon_ntff_profile_hook = lambda h: None
    sys.modules["antenv.axon_hooks"] = mod


def run(x, y_s, weight, bias, trace=False, tmpdir=None):
    nc = _get_nc()
    if trace:
        _install_trace_support()
    in_maps = _prep_inputs(x, y_s, weight, bias)
    res = run_bass_kernel_spmd(
        nc, in_maps, core_ids=list(range(B)), trace=trace, tmpdir=tmpdir
    )
    out = np.stack(
        [res.results[b]["out"].reshape(CO, H, W) for b in range(B)]
    ).astype(np.float32)
    return out, res


def kernel(x, y_s, weight, bias):
    out, _ = run(
        np.asarray(x, dtype=np.float32),
        np.asarray(y_s, dtype=np.float32),
        np.asarray(weight, dtype=np.float32),
        np.asarray(bias, dtype=np.float32),
    )
    return out
